# revision 29
# baseline (speedup 1.0000x reference)
"""EnvLSTM Trainium2 kernel (nn_EnvLSTM_86577950753068).

Bidirectional gated (LSTM-like) scan over T=4096 steps, MEM=1024, solved by
time-parallel gate-Picard iteration:
  - 8 cores: 0-3 left scan (time blocks 0-3), 4-7 right scan (reversed input,
    block-paired so core 4+j holds original-time block j).
  - Per sweep: gate preacts A updated incrementally (A += U @ delta in bf16)
    or refreshed exactly (A = WX + U @ C, f32r matmuls), sigmoid/tanh on ACT,
    and the affine recurrence c_{t+1} = f_t c_t + i_t g_t solved with the
    native DVE tensor_tensor_scan. Block boundary states travel via a 4KB
    AllGather per sweep (one-sweep delay, validated numerically).
  - The init (WX) / refresh (U@C) and output-projection matmuls run in
    float32r (fp22 operand reads, 4x fp32 throughput; rel err ~1.6e-4).
  - Output projection tanh(wo@lenv + uo@renv + bo) after a pairwise env
    AllGather, split by columns: left core j computes out[:, 0:512] and
    right core j+4 computes out[:, 512:1024] of time block j (per-core
    wouoT/bo halves keep the SPMD program uniform); host concatenates.
  - 14-sweep schedule (p1 + 8r + f + 4r) lands at rel err ~5e-3 against the
    fp32 reference (gate: 2e-2).
"""

import os
import numpy as np
import ml_dtypes

import concourse.bass as bass
import concourse.mybir as mybir
import concourse.tile as tile
from concourse.bass_utils import run_bass_kernel_spmd

F32 = mybir.dt.float32
F32R = mybir.dt.float32r
BF16 = mybir.dt.bfloat16
AF = mybir.ActivationFunctionType
ALU = mybir.AluOpType

T, IN, MEM, OUT = 4096, 1024, 1024, 1024
TC = 1024              # time steps per core
P = 128                # partitions
NK = 8                 # contraction k-tiles (MEM/128)
NGT = 24               # gate tiles (3*MEM/128)
NTCH = 2               # t chunks of 512 per core
NCORES = 8

# sweep schedule: 'p1' = init (A = WX + b, C from zero guess),
# 'r' = residual bf16 sweep, 'f' = full fp32 refresh sweep.
# 14 sweeps land at rel err ~5e-3 (gate is 2e-2); the bf16 sweeps before the
# final fp32 refresh do the bulk of the convergence.
SCHEDULE = (['p1'] + ['r'] * 8 + ['f'] + ['r'] * 4)



def _tctile(tc, shape, dtype, name):
    t, _free = tc.tile(shape, dtype, name=name)
    return t


def legalize_waits(nc, max_w=1, max_u=1, max_w_engine=1):
    """This toolchain's walrus accepts at most one sync wait/update per
    instruction; split extras onto same-engine NoOp carriers (engine program
    order preserves semantics)."""
    n_nops = 0
    for bb in nc.main_func.blocks:
        out = []
        for ins in bb.instructions:
            si = ins.sync_info
            if si is None:
                out.append(ins)
                continue
            waits = list(si.on_wait or [])
            ups = list(si.on_update or [])
            cap = max_w if isinstance(ins, mybir.InstDMACopy) else max_w_engine
            pre, post = [], []
            if len(waits) > cap:
                for w in waits[:-cap]:
                    nop = mybir.InstNoOp(name=f"{ins.name}_lw{n_nops}", ins=[], outs=[])
                    n_nops += 1
                    nop.engine = ins.engine
                    nop.sync_info = mybir.SyncInfo(on_wait=[w], on_update=[])
                    pre.append(nop)
                waits = waits[-cap:]
            if len(ups) > max_u:
                for u in ups[max_u:]:
                    nop = mybir.InstNoOp(name=f"{ins.name}_lu{n_nops}", ins=[], outs=[])
                    n_nops += 1
                    nop.engine = ins.engine
                    nop.sync_info = mybir.SyncInfo(on_wait=[], on_update=[u])
                    post.append(nop)
                ups = ups[:max_u]
            ins.sync_info = mybir.SyncInfo(on_wait=waits, on_update=ups)
            out.extend(pre)
            out.append(ins)
            out.extend(post)
        bb.instructions = out
    return n_nops


def build_kernel(schedule):
    nc = bass.Bass("TRN2", target_bir_lowering=False, debug=False,
                   num_devices=1 if os.environ.get("KERNEL_SIM_NOCC", "0") == "1"
                   else NCORES)

    dt_in = nc.dram_tensor("xT", [IN, TC], F32R, kind="ExternalInput").ap()
    dt_wT = nc.dram_tensor("w_allT", [IN, 3 * MEM], F32R, kind="ExternalInput").ap()
    dt_uT = nc.dram_tensor("u_allT", [MEM, 3 * MEM], F32R, kind="ExternalInput").ap()
    dt_ubf = nc.dram_tensor("u_bf", [MEM, 3 * MEM], BF16, kind="ExternalInput").ap()
    # per-core half of the output weights (left cores cols 0:512, right 512:)
    dt_wouo = nc.dram_tensor("wouoT", [2, MEM, 512], F32R, kind="ExternalInput").ap()
    dt_consts = nc.dram_tensor("consts", [P, 4224], F32, kind="ExternalInput").ap()
    dt_sel = nc.dram_tensor("sel", [P, 8], F32, kind="ExternalInput").ap()
    dt_c0v = nc.dram_tensor("c0_vec", [P, 8], F32, kind="ExternalInput").ap()
    dt_c0h = nc.dram_tensor("c0_head", [P, 8], F32, kind="ExternalInput").ap()
    dt_out = nc.dram_tensor("out_part", [TC, 512], F32, kind="ExternalOutput").ap()

    S = len(schedule)

    with tile.TileContext(nc) as tc:
        # ---- persistent SBUF (one bufs=1 pool, unique tags -> unique slots) ----
        with tc.tile_pool(name="persist", bufs=1) as persist:
            u_sb = persist.tile([P, NK, 3 * MEM], BF16, name="u_sb")
            c_a = persist.tile([P, NK, TC + 1], F32, name="c_a")
            c_b = persist.tile([P, NK, TC + 1], F32, name="c_b")
            u_buf = persist.tile([P, NK, TC + 1], F32, name="u_buf")
            ci_a = persist.tile([P, NK], F32, name="ci_a")
            ci_b = persist.tile([P, NK], F32, name="ci_b")
            sel_sb = persist.tile([P, 8], F32, name="sel_sb")
            c0v_sb = persist.tile([P, NK], F32, name="c0v_sb")
            c0h_sb = persist.tile([P, NK], F32, name="c0h_sb")
            bnd_all = persist.tile([P, NK, 8], F32, name="bnd_all")
            bnd_tmp = persist.tile([P, NK, 8], F32, name="bnd_tmp")
            const_sb = persist.tile([P, 4224], F32, name="const_sb")
            colsave = persist.tile([P, NK], F32, name="colsave")
            _body(nc, tc, locals())
    return nc


def _body(nc, tc, env):
    u_sb = env["u_sb"]; c_a = env["c_a"]; c_b = env["c_b"]; u_buf = env["u_buf"]
    ci_a = env["ci_a"]; ci_b = env["ci_b"]; sel_sb = env["sel_sb"]
    c0v_sb = env["c0v_sb"]; c0h_sb = env["c0h_sb"]; bnd_all = env["bnd_all"]
    bnd_tmp = env["bnd_tmp"]; const_sb = env["const_sb"]
    colsave = env["colsave"]
    schedule = env["schedule"]
    dt_in = env["dt_in"]; dt_wT = env["dt_wT"]; dt_uT = env["dt_uT"]
    dt_ubf = env["dt_ubf"]; dt_wouo = env["dt_wouo"]; dt_consts = env["dt_consts"]
    dt_sel = env["dt_sel"]; dt_c0v = env["dt_c0v"]; dt_c0h = env["dt_c0h"]
    dt_out = env["dt_out"]
    S = len(schedule)
    if True:
        # ---- DRAM internals ----
        with tc.tile_pool(name="dram", bufs=1, space="DRAM") as dram:
            wx_dram = dram.tile([3 * MEM, TC], F32, name="wx_dram")
            a_dram_a = dram.tile([3 * MEM, TC], F32, name="a_dram_a")
            a_dram_b = dram.tile([3 * MEM, TC], F32, name="a_dram_b")
            a_bf_a = dram.tile([3 * MEM, TC], BF16, name="a_bf_a")
            a_bf_b = dram.tile([3 * MEM, TC], BF16, name="a_bf_b")
            bnd_in = dram.tile([1, MEM], F32, name="bnd_in")
            bnd_out = dram.tile([NCORES, MEM], F32, name="bnd_out")
            env_in = dram.tile([MEM, TC], F32, name="env_in")
            env_out = dram.tile([2, MEM, TC], F32, name="env_out")

            with (
                tc.tile_pool(name="psum", bufs=8, space="PSUM") as psum_pool,
                tc.tile_pool(name="stage_a", bufs=10) as stage_a,     # fp32 [128,512]
                tc.tile_pool(name="stage_b", bufs=12) as stage_b,    # fp32/bf16 [128,512]
                tc.tile_pool(name="stage_w", bufs=8) as stage_w,     # lhsT [128,128]
                tc.tile_pool(name="stage_ig", bufs=3) as stage_ig,   # i/g act [128,512]
            ):
                # ---- load persistent inputs ----
                nc.sync.dma_start(
                    u_sb[:, :, :],
                    dt_ubf.rearrange("(k p) g -> p k g", p=P))
                nc.sync.dma_start(sel_sb[:, :], dt_sel)
                nc.sync.dma_start(c0v_sb[:, :], dt_c0v)
                nc.sync.dma_start(c0h_sb[:, :], dt_c0h)
                nc.sync.dma_start(const_sb[:, :], dt_consts)

                nc.gpsimd.memset(c_b[:, :, :], 0.0)
                nc.gpsimd.memset(u_buf[:, :, 0], 0.0)
                nc.vector.tensor_copy(ci_a[:, :], c0v_sb[:, :])

                cbufs = [c_a, c_b]
                cins = [ci_a, ci_b]
                abufs = [a_dram_a, a_dram_b]
                abfbufs = [a_bf_a, a_bf_b]
                first_f = schedule.index('f') if 'f' in schedule else S
                # timing-only phase isolation (results invalid when set)
                skip_mm = os.environ.get("KERNEL_SKIP_MM", "0") == "1"
                skip_scan = os.environ.get("KERNEL_SKIP_SCAN", "0") == "1"
                skip_bnd = os.environ.get("KERNEL_SKIP_BND", "0") == "1"
                no_ag = os.environ.get("KERNEL_NO_AG", "0") == "1"

                for s, mode in enumerate(schedule):
                    c_next = cbufs[s % 2]      # scan output of sweep s
                    c_cur = cbufs[(s + 1) % 2]  # C_{s-1}
                    c_in = cins[s % 2]
                    c_in_nxt = cins[(s + 1) % 2]

                    if mode == 'r':
                        # col 512 of C_{s-2} gets overwritten by tch-0 f-gate
                        # writes before tch-1 deltas read it; snapshot it.
                        nc.vector.tensor_copy(colsave[:, :], c_next[:, :, 512])

                    for tch in range(NTCH) if not skip_mm else []:
                        t0 = tch * 512
                        # rhs tiles for this t-chunk
                        if mode == 'p1':
                            rhs_tiles = []
                            for k in range(NK):
                                xt = stage_a.tile([P, 512], F32R, name=f"xt{s}_{tch}_{k}",
                                                  tag="stage_a")
                                nc.sync.dma_start(
                                    xt[:, :], dt_in[k * P:(k + 1) * P, t0:t0 + 512])
                                rhs_tiles.append(xt)
                        elif mode == 'r':
                            rhs_tiles = []
                            for k in range(NK):
                                dtl = stage_b.tile([P, 512], BF16, name=f"d{s}_{tch}_{k}",
                                                   tag="stage_b")
                                if tch == 0:
                                    nc.vector.tensor_sub(
                                        dtl[:, :],
                                        c_cur[:, k, t0:t0 + 512],
                                        c_next[:, k, t0:t0 + 512])
                                else:
                                    # col 512 from snapshot, rest from buffer
                                    nc.vector.tensor_sub(
                                        dtl[:, 0:1],
                                        c_cur[:, k, 512:513],
                                        colsave[:, k:k + 1])
                                    nc.vector.tensor_sub(
                                        dtl[:, 1:512],
                                        c_cur[:, k, 513:1024],
                                        c_next[:, k, 513:1024])
                                rhs_tiles.append(dtl)
                        else:  # refresh: relabel C_{s-1} slices to f32r via DMA
                            rhs_tiles = []
                            for k in range(NK):
                                cr = stage_a.tile([P, 512], F32R,
                                                  name=f"cr{s}_{tch}_{k}",
                                                  tag="stage_a")
                                nc.sync.dma_start(
                                    cr[:, :],
                                    c_cur[:, k, t0:t0 + 512].bitcast(F32R))
                                rhs_tiles.append(cr)

                        sti_tiles = {}
                        for j in range(NK):
                            for gate in range(3):
                                gt = gate * NK + j
                                g0 = gt * P
                                ps = psum_pool.tile([P, 512], F32,
                                                    name=f"ps{s}_{tch}_{gt}", tag="ps")
                                # matmul accumulation
                                for k in range(NK):
                                    if mode == 'p1':
                                        wt = stage_w.tile([P, P], F32R,
                                                          name=f"w{s}_{tch}_{gt}_{k}",
                                                          tag="stage_w")
                                        nc.sync.dma_start(
                                            wt[:, :],
                                            dt_wT[k * P:(k + 1) * P, g0:g0 + P])
                                        lhsT = wt[:, :]
                                        rhs = rhs_tiles[k][:, :]
                                    elif mode == 'r':
                                        lhsT = u_sb[:, k, g0:g0 + P]
                                        rhs = rhs_tiles[k][:, :]
                                    else:
                                        ut = stage_w.tile([P, P], F32R,
                                                          name=f"u{s}_{tch}_{gt}_{k}",
                                                          tag="stage_w")
                                        nc.sync.dma_start(
                                            ut[:, :],
                                            dt_uT[k * P:(k + 1) * P, g0:g0 + P])
                                        lhsT = ut[:, :]
                                        rhs = rhs_tiles[k][:, :]
                                    nc.tensor.matmul(
                                        ps[:, :], lhsT, rhs,
                                        start=(k == 0),
                                        stop=(k == NK - 1 and mode != 'p1'))
                                if mode == 'p1':
                                    # + bias via K=1 matmul
                                    nc.tensor.matmul(
                                        ps[:, :],
                                        const_sb[0:1, g0:g0 + P],
                                        const_sb[0:1, 3712:4224],
                                        start=False, stop=True)

                                # A update
                                if mode == 'p1':
                                    a_new = stage_a.tile([P, 512], F32,
                                                         name=f"an{s}_{tch}_{gt}",
                                                         tag="stage_a")
                                    nc.vector.tensor_copy(a_new[:, :], ps[:, :])
                                    nc.sync.dma_start(
                                        wx_dram[g0:g0 + P, t0:t0 + 512], a_new[:, :])
                                    act_src = a_new
                                else:
                                    # early residual sweeps (error >> 1e-2)
                                    # stream A in bf16: half the DMA bytes
                                    early = mode == 'r' and s < first_f
                                    if mode == 'f' or s == 1:
                                        a_old = stage_a.tile([P, 512], F32,
                                                             name=f"ao{s}_{tch}_{gt}",
                                                             tag="stage_a")
                                        nc.sync.dma_start(
                                            a_old[:, :],
                                            wx_dram[g0:g0 + P, t0:t0 + 512])
                                    elif early:
                                        a_old = stage_b.tile([P, 512], BF16,
                                                             name=f"ao{s}_{tch}_{gt}",
                                                             tag="stage_b")
                                        nc.sync.dma_start(
                                            a_old[:, :],
                                            abfbufs[s % 2][g0:g0 + P, t0:t0 + 512])
                                    else:
                                        a_old = stage_a.tile([P, 512], F32,
                                                             name=f"ao{s}_{tch}_{gt}",
                                                             tag="stage_a")
                                        nc.sync.dma_start(
                                            a_old[:, :],
                                            abufs[s % 2][g0:g0 + P, t0:t0 + 512])
                                    if early:
                                        a_new = stage_b.tile([P, 512], BF16,
                                                             name=f"an{s}_{tch}_{gt}",
                                                             tag="stage_b")
                                    else:
                                        a_new = stage_a.tile([P, 512], F32,
                                                             name=f"an{s}_{tch}_{gt}",
                                                             tag="stage_a")
                                    nc.vector.tensor_add(a_new[:, :], ps[:, :],
                                                         a_old[:, :])
                                    # skip write-back when the next sweep is a
                                    # refresh (it reads WX) or this is the last
                                    nxt_reads_a = s + 1 < S and schedule[s + 1] == 'r'
                                    if nxt_reads_a:
                                        dst = abfbufs[(s + 1) % 2] if s + 1 < first_f \
                                            else abufs[(s + 1) % 2]
                                        if (s + 1 < first_f) != early and not (mode == 'f' or s == 1):
                                            pass
                                        nc.scalar.dma_start(
                                            dst[g0:g0 + P, t0:t0 + 512],
                                            a_new[:, :])
                                    act_src = a_new

                                # activations
                                dst_sl = (1 + t0, 512)
                                if gate == 0:   # forget gate -> scan data0 (in C_next)
                                    nc.scalar.activation(
                                        c_next[:, j, dst_sl[0]:dst_sl[0] + 512],
                                        act_src[:, :], AF.Sigmoid)
                                elif gate == 1:  # input gate
                                    sti = stage_ig.tile([P, 512], F32,
                                                        name=f"i{s}_{tch}_{j}",
                                                        tag="stage_ig")
                                    nc.scalar.activation(sti[:, :], act_src[:, :],
                                                         AF.Sigmoid)
                                    sti_tiles[j] = sti
                                else:            # candidate -> u = i*g
                                    stg = stage_ig.tile([P, 512], F32,
                                                        name=f"g{s}_{tch}_{j}",
                                                        tag="stage_ig")
                                    nc.scalar.activation(stg[:, :], act_src[:, :],
                                                         AF.Tanh)
                                    nc.vector.tensor_mul(
                                        u_buf[:, j, dst_sl[0]:dst_sl[0] + 512],
                                        sti_tiles[j][:, :], stg[:, :])

                    # ---- scan phase ----
                    if not skip_scan:
                        nc.gpsimd.memset(c_next[:, :, 0], 1.0)  # identity step coeff
                        for j in range(NK):
                            nc.vector.tensor_tensor_scan(
                                c_next[:, j, :],
                                c_next[:, j, :],
                                u_buf[:, j, :],
                                c_in[:, j:j + 1],
                                ALU.mult, ALU.add)

                    # ---- boundary exchange (skip on last sweep) ----
                    if s < S - 1 and skip_bnd:
                        # timing-only: keep c_in written, skip DMA/AG path
                        nc.vector.tensor_copy(ci_b[:, :] if (s + 1) % 2 else ci_a[:, :],
                                              c0v_sb[:, :])
                    if s < S - 1 and not skip_bnd:
                        nc.sync.dma_start(
                            bnd_in.rearrange("o (j p) -> p (o j)", p=P),
                            c_next[:, :, TC])
                        if os.environ.get("KERNEL_SIM_NOCC", "0") != "1" and not no_ag:
                            nc.gpsimd.collective_compute(
                                "AllGather", ALU.bypass,
                                replica_groups=[list(range(NCORES))],
                                ins=[bnd_in.opt()],
                                outs=[bnd_out.opt()])
                        for r in range(NCORES):
                            nc.sync.dma_start(
                                bnd_all[:, :, r],
                                bnd_out[r:r + 1, :].rearrange(
                                    "o (j p) -> p (o j)", p=P))
                        nc.vector.tensor_mul(
                            bnd_tmp[:, :, :], bnd_all[:, :, :],
                            sel_sb[:, :].unsqueeze(1).broadcast_to([P, NK, 8]))
                        nc.vector.tensor_reduce(
                            c_in_nxt[:, :].unsqueeze(2), bnd_tmp[:, :, :],
                            op=ALU.add, axis=mybir.AxisListType.X)
                        nc.vector.tensor_add(c_in_nxt[:, :], c_in_nxt[:, :],
                                             c0h_sb[:, :])

                # ---- output stage ----
                if os.environ.get("KERNEL_SIM_NOOUT", "0") == "1":
                    return
                c_fin = cbufs[(S - 1) % 2]
                nc.sync.dma_start(
                    env_in.rearrange("(j p) t -> p j t", p=P),
                    c_fin[:, :, 0:TC])
                if os.environ.get("KERNEL_SIM_NOCC", "0") != "1":
                    nc.gpsimd.collective_compute(
                        "AllGather", ALU.bypass,
                        replica_groups=[[0, 4], [1, 5], [2, 6], [3, 7]],
                        ins=[env_in.opt()],
                        outs=[env_out.opt()])

                # each core computes only its half of the output columns; the
                # host pairs left core j (cols 0:512) with right core j+4
                # (cols 512:1024). dt_wouo/consts carry the per-core half.
                pso = [psum_pool.tile([P, 512], F32, name=f"po_{i}", tag="ps")
                       for i in range(8)]
                first = True
                for (slot, wsl) in ((0, 0), (1, 1)):
                    for mk in range(NK):
                        wt = stage_a.tile([P, 512], F32R, name=f"ow_{slot}_{mk}",
                                          tag="stage_a")
                        nc.sync.dma_start(
                            wt[:, :],
                            dt_wouo[wsl, mk * P:(mk + 1) * P, :])
                        for half in range(2):
                            # slot 1 holds the right scan in reversed-time
                            # (local) order: original half h lives in local
                            # half (1-h), column-reversed.
                            lhalf = half if slot == 0 else 1 - half
                            if slot == 0:
                                esu = stage_a.tile([P, 512], F32R,
                                                   name=f"oe_{slot}_{mk}_{half}",
                                                   tag="stage_a")
                                nc.sync.dma_start(
                                    esu[:, :],
                                    env_out[slot, mk * P:(mk + 1) * P,
                                            lhalf * 512:(lhalf + 1) * 512]
                                    .bitcast(F32R))
                            else:
                                es = stage_a.tile([P, 512], F32,
                                                  name=f"oe_{slot}_{mk}_{half}",
                                                  tag="stage_a")
                                nc.sync.dma_start(
                                    es[:, :],
                                    env_out[slot, mk * P:(mk + 1) * P,
                                            lhalf * 512:(lhalf + 1) * 512])
                                esf = stage_a.tile([P, 512], F32,
                                                   name=f"oer_{slot}_{mk}_{half}",
                                                   tag="stage_a")
                                nc.vector.tensor_copy(esf[:, :], es[:, ::-1])
                                esu = stage_a.tile([P, 512], F32R,
                                                   name=f"oeu_{slot}_{mk}_{half}",
                                                   tag="stage_a")
                                nc.sync.dma_start(esu[:, :],
                                                  esf[:, :].bitcast(F32R))
                            for q in range(4):
                                i = half * 4 + q
                                nc.tensor.matmul(pso[i][:, :],
                                                 esu[:, q * P:(q + 1) * P],
                                                 wt[:, :],
                                                 start=first and mk == 0,
                                                 stop=False)
                    first = False
                for i in range(8):
                    nc.tensor.matmul(pso[i][:, :], const_sb[0:1, 3584:3712],
                                     const_sb[0:1, 3072:3584],
                                     start=False, stop=True)
                    ot = stage_a.tile([P, 512], F32, name=f"oo_{i}",
                                      tag="stage_a")
                    nc.scalar.activation(ot[:, :], pso[i][:, :], AF.Tanh)
                    nc.sync.dma_start(
                        dt_out[i * P:(i + 1) * P, :], ot[:, :])



def _prep_inputs(inputs):
    """Build the 8 per-core input maps from the full problem inputs."""
    bf = ml_dtypes.bfloat16
    x = np.ascontiguousarray(inputs["x"], dtype=np.float32)
    maps = []

    wouoT_full = np.stack([np.ascontiguousarray(inputs["wo"].T),
                           np.ascontiguousarray(inputs["uo"].T)]).astype(np.float32)
    bo = np.ascontiguousarray(inputs["bo"], np.float32).reshape(OUT)

    for c in range(NCORES):
        side = "l" if c < 4 else "r"
        w_all = np.concatenate([inputs[f"w{g}_{side}"] for g in ("f", "i", "c")], 0)
        u_all = np.concatenate([inputs[f"u{g}_{side}"] for g in ("f", "i", "c")], 0)
        b_all = np.concatenate([inputs[f"b{g}_{side}"] for g in ("f", "i", "c")], 0)
        c0 = np.asarray(inputs[f"c0_{side}"], np.float32)

        if c < 4:
            blk = c
            x_loc = x[TC * blk: TC * (blk + 1)]
            prev = c - 1 if c > 0 else None
            head = c == 0
        else:
            j = c - 4
            blk = j
            x_loc = x[TC * j: TC * (j + 1)][::-1]
            prev = c + 1 if j < 3 else None
            head = c == 7

        sel = np.zeros((P, 8), np.float32)
        if prev is not None:
            sel[:, prev] = 1.0
        c0_vec = np.ascontiguousarray(c0.reshape(NK, P).T, np.float32)
        c0_head = c0_vec if head else np.zeros_like(c0_vec)

        o0 = 0 if c < 4 else 512
        crow = np.concatenate([b_all.astype(np.float32), bo[o0:o0 + 512],
                               np.ones(P, np.float32),
                               np.ones(512, np.float32)])
        consts = np.broadcast_to(crow, (P, 4224)).copy()
        maps.append({
            "xT": np.ascontiguousarray(x_loc.T, np.float32),
            "w_allT": np.ascontiguousarray(w_all.T, np.float32),
            "u_allT": np.ascontiguousarray(u_all.T, np.float32),
            "u_bf": np.ascontiguousarray(u_all.T.astype(bf)),
            "wouoT": np.ascontiguousarray(wouoT_full[:, :, o0:o0 + 512]),
            "consts": consts,
            "sel": sel,
            "c0_vec": c0_vec,
            "c0_head": c0_head,
        })
    return maps


_CACHED = {}


def kernel(**inputs) -> np.ndarray:
    _CACHED["inputs"] = inputs
    if os.environ.get("KERNEL_SPMD_PATH", "0") == "1":
        in_maps = _prep_inputs(inputs)
        nc = build_kernel(SCHEDULE)
        legalize_waits(nc)
        res = run_bass_kernel_spmd(nc, in_maps, core_ids=list(range(NCORES)),
                                   trace=False)
        results = res.results
    else:
        results, times = timed_run(n_iters=80)
        _CACHED["times"] = times
    out = np.concatenate(
        [np.concatenate([results[c]["out_part"], results[c + 4]["out_part"]],
                        axis=1) for c in range(4)], axis=0)
    return out.astype(np.float32)


if __name__ == "__main__":
    nc = build_kernel(SCHEDULE)
    print("built ok; instructions:",
          sum(len(b.instructions) for b in nc.main_func.blocks))


def timed_run(n_iters=3):
    """Compile once, keep inputs device-resident, time executions.

    Returns (results_for_cores, [per-iter seconds])."""
    import time
    import jax
    from jax.sharding import Mesh, PartitionSpec, NamedSharding
    from jax.experimental.shard_map import shard_map
    from concourse import bass2jax

    inputs = _CACHED["inputs"]
    in_maps = _prep_inputs(inputs)
    nc = build_kernel(SCHEDULE)
    legalize_waits(nc)
    bass2jax.install_neuronx_cc_hook()

    partition_name = nc.partition_id_tensor.name if nc.partition_id_tensor else None
    in_names, out_names, out_avals, zero_outs = [], [], [], []
    import concourse.mybir as mybir_
    for alloc in nc.m.functions[0].allocations:
        if not isinstance(alloc, mybir_.MemoryLocationSet):
            continue
        name = alloc.memorylocations[0].name
        if alloc.kind == "ExternalInput":
            if name != partition_name:
                in_names.append(name)
        elif alloc.kind == "ExternalOutput":
            shape = tuple(alloc.tensor_shape)
            dtype = mybir_.dt.np(alloc.dtype)
            out_names.append(name)
            out_avals.append(jax.core.ShapedArray(shape, dtype))
            zero_outs.append(np.zeros(shape, dtype))
    n_params = len(in_names)
    all_in_names = list(in_names) + list(out_names)
    if partition_name is not None:
        all_in_names.append(partition_name)

    def _body(*args):
        operands = list(args)
        if partition_name is not None:
            operands.append(bass2jax.partition_id_tensor())
        outs = bass2jax._bass_exec_p.bind(
            *operands,
            out_avals=tuple(out_avals),
            in_names=tuple(all_in_names),
            out_names=tuple(out_names),
            lowering_input_output_aliases=(),
            sim_require_finite=True,
            sim_require_nnan=True,
            nc=nc,
        )
        return tuple(outs)

    devices = jax.devices()[:NCORES]
    mesh = Mesh(np.asarray(devices), ("core",))
    in_specs = (PartitionSpec("core"),) * (n_params + len(out_names))
    out_specs = (PartitionSpec("core"),) * len(out_names)
    fn = jax.jit(
        shard_map(_body, mesh=mesh, in_specs=in_specs, out_specs=out_specs,
                  check_rep=False),
        keep_unused=True,
    )
    concat_in = [
        np.concatenate([np.asarray(in_maps[c][nm])[None] for c in range(NCORES)],
                       axis=0).reshape(-1, *np.asarray(in_maps[0][nm]).shape[1:])
        for nm in in_names
    ]
    sh = NamedSharding(mesh, PartitionSpec("core"))
    dev_in = [jax.device_put(a, sh) for a in concat_in]
    dev_zero = [jax.device_put(
        np.zeros((NCORES * z.shape[0],) + z.shape[1:], z.dtype), sh)
        for z in zero_outs]
    times = []
    out_arrs = None
    for i in range(n_iters):
        t0 = time.time()
        out_arrs = fn(*dev_in, *dev_zero)
        jax.block_until_ready(out_arrs)
        times.append(time.time() - t0)
    results = [
        {nm: np.asarray(out_arrs[i]).reshape(NCORES, *out_avals[i].shape)[c]
         for i, nm in enumerate(out_names)}
        for c in range(NCORES)
    ]
    return results, times

